# revision 1
# baseline (speedup 1.0000x reference)
"""Trainium2 Bass kernel for nn_ExpertGroup (moe_routing).

Sharding: 8 cores = (batch b in 0..3) x (seq half j in 0..1); each core owns
1024 tokens. Activations flow in transposed [feature, token] layout so every
matmul contracts over the partition dim. The sequence-mixing adapt attention
needs full-S adapt_in/adapt_out, so paired cores AllGather their N-layout
halves (one collective carrying both tensors), overlapped with the
independent expert branch. Matmul operands are bf16 (host-cast weights),
accumulation f32 in PSUM.
"""

import numpy as np
import ml_dtypes

import concourse.bacc as bacc
import concourse.mybir as mybir
import concourse.tile as tile
from concourse import bass_utils

F32 = mybir.dt.float32
BF16 = mybir.dt.bfloat16
AX = mybir.AxisListType
OP = mybir.AluOpType
AF = mybir.ActivationFunctionType

B, S, D, H, AD, E = 4, 2048, 1024, 2048, 128, 8
PHASES = []


def _mark(nc, name):
    PHASES.append((name, nc.next_id()))

TOK = 1024          # tokens per core
N_CORES = 8
NCH = TOK // 512    # 512-wide matmul chunks of the own token range
BF = ml_dtypes.bfloat16

_NC_CACHE = None


def build(fake_cc=False):
    nc = bacc.Bacc("TRN2", target_bir_lowering=False, debug=False,
                   num_devices=N_CORES)

    # ---- per-core DRAM parameters ----
    xt = nc.declare_dram_parameter("xt", [D, TOK], BF16, isOutput=False)
    ew = nc.declare_dram_parameter("ew", [TOK, E], F32, isOutput=False)
    ewt = nc.declare_dram_parameter("ewt", [E, TOK], F32, isOutput=False)
    wu_t = nc.declare_dram_parameter("wu_t", [16, 128, 8, 128], BF16, isOutput=False)
    wg_t = nc.declare_dram_parameter("wg_t", [16, 128, 8, 128], BF16, isOutput=False)
    wd_t = nc.declare_dram_parameter("wd_t", [8, 128, 16, 128], BF16, isOutput=False)
    wo_t = nc.declare_dram_parameter("wo_t", [H, D], BF16, isOutput=False)
    wpre_t = nc.declare_dram_parameter("wpre_t", [D, AD], BF16, isOutput=False)
    wpost_t = nc.declare_dram_parameter("wpost_t", [H, AD], BF16, isOutput=False)
    wap_t = nc.declare_dram_parameter("wap_t", [AD, H], BF16, isOutput=False)  # 0.1 folded
    wp = nc.declare_dram_parameter("wp", [H, AD], BF16, isOutput=False)        # 0.1 folded
    a_t = nc.declare_dram_parameter("a_t", [E, AD, AD], BF16, isOutput=False)
    bu = nc.declare_dram_parameter("bu", [H], F32, isOutput=False)
    bg = nc.declare_dram_parameter("bg", [H], F32, isOutput=False)
    bd = nc.declare_dram_parameter("bd", [D], F32, isOutput=False)
    bpre = nc.declare_dram_parameter("bpre", [AD], F32, isOutput=False)
    bpost = nc.declare_dram_parameter("bpost", [AD], F32, isOutput=False)
    ln_g = nc.declare_dram_parameter("ln_g", [AD], F32, isOutput=False)
    ln_b = nc.declare_dram_parameter("ln_b", [AD], F32, isOutput=False)
    eg = nc.declare_dram_parameter("eg", [E, AD], F32, isOutput=False)
    eb = nc.declare_dram_parameter("eb", [E, AD], F32, isOutput=False)
    id_bf = nc.declare_dram_parameter("id_bf", [128, 128], BF16, isOutput=False)
    id_f32 = nc.declare_dram_parameter("id_f32", [128, 128], F32, isOutput=False)
    out = nc.declare_dram_parameter("out", [D, TOK], F32, isOutput=True)

    with tile.TileContext(nc) as tc:
        _emit(nc, tc, locals(), fake_cc)
    nc.compile()
    return nc


def _emit(nc, tc, P, fake_cc=False):
    xt, ew, ewt = P["xt"], P["ew"], P["ewt"]
    wu_t, wg_t, wd_t, wo_t = P["wu_t"], P["wg_t"], P["wd_t"], P["wo_t"]
    wpre_t, wpost_t, wap_t, wp, a_t = (
        P["wpre_t"], P["wpost_t"], P["wap_t"], P["wp"], P["a_t"])
    bu, bg, bd, bpre, bpost = P["bu"], P["bg"], P["bd"], P["bpre"], P["bpost"]
    ln_g, ln_b, eg, eb = P["ln_g"], P["ln_b"], P["eg"], P["eb"]
    id_bf, id_f32, out = P["id_bf"], P["id_f32"], P["out"]

    ctx = tc  # alias

    import contextlib
    stack = contextlib.ExitStack()
    pool = stack.enter_context(tc.tile_pool(name="res", bufs=1))
    scr = stack.enter_context(tc.tile_pool(name="scr", bufs=2))
    wpool = stack.enter_context(tc.tile_pool(name="wts", bufs=2))
    ps = stack.enter_context(tc.tile_pool(name="ps", bufs=2, space="PSUM"))
    dram = stack.enter_context(tc.tile_pool(name="dram", bufs=1, space="DRAM"))

    # =================== P0: constants / small prep ===================
    _mark(nc, "P0")
    xt_sb = pool.tile([128, 8, TOK], BF16, tag="xt_sb")
    nc.sync.dma_start(xt_sb[:], xt.ap().rearrange("(k p) s -> p k s", p=128))
    wpre_sb = pool.tile([128, 8, AD], BF16, tag="wpre_sb")
    nc.sync.dma_start(wpre_sb[:], wpre_t.ap().rearrange("(k p) a -> p k a", p=128))
    ident_b = pool.tile([128, 128], BF16, tag="ident_b")
    ident_f = pool.tile([128, 128], F32, tag="ident_f")
    nc.sync.dma_start(ident_b[:], id_bf[:])
    nc.sync.dma_start(ident_f[:], id_f32[:])

    but = pool.tile([128, 16], F32, tag="but")
    bgt = pool.tile([128, 16], F32, tag="bgt")
    bdt = pool.tile([128, 8], F32, tag="bdt")
    nc.sync.dma_start(but[:], bu.ap().rearrange("(t p) -> p t", p=128))
    nc.sync.dma_start(bgt[:], bg.ap().rearrange("(t p) -> p t", p=128))
    nc.sync.dma_start(bdt[:], bd.ap().rearrange("(t p) -> p t", p=128))
    bpre_c = pool.tile([128, 1], F32, tag="bpre_c")
    bpost_c = pool.tile([128, 1], F32, tag="bpost_c")
    nc.sync.dma_start(bpre_c[:], bpre.ap().unsqueeze(1))
    nc.sync.dma_start(bpost_c[:], bpost.ap().unsqueeze(1))

    lng_c = pool.tile([128, 1], F32, tag="lng_c")
    lnb_c = pool.tile([128, 1], F32, tag="lnb_c")
    nc.sync.dma_start(lng_c[:], ln_g.ap().unsqueeze(1))
    nc.sync.dma_start(lnb_c[:], ln_b.ap().unsqueeze(1))
    lngr = pool.tile([1, 128], F32, tag="lngr")
    lnbr = pool.tile([1, 128], F32, tag="lnbr")
    nc.sync.dma_start(lngr[:], ln_g.ap().unsqueeze(0))
    nc.sync.dma_start(lnbr[:], ln_b.ap().unsqueeze(0))
    gB = pool.tile([128, 128], F32, tag="gB")
    bB = pool.tile([128, 128], F32, tag="bB")
    nc.gpsimd.partition_broadcast(gB[:], lngr[:])
    nc.gpsimd.partition_broadcast(bB[:], lnbr[:])

    egr = pool.tile([1, E * AD], F32, tag="egr")
    nc.sync.dma_start(egr[:], eg.ap().rearrange("e a -> (e a)").unsqueeze(0))
    egr_bf = pool.tile([1, E * AD], BF16, tag="egr_bf")
    nc.vector.tensor_copy(egr_bf[:], egr[:])
    egB = pool.tile([128, E, AD], BF16, tag="egB")
    for e in range(E):
        nc.gpsimd.partition_broadcast(egB[:, e, :], egr_bf[:, e * AD:(e + 1) * AD])
    eb_f32 = pool.tile([E, AD], F32, tag="eb_f32")
    nc.sync.dma_start(eb_f32[:], eb.ap())
    eb_nat = pool.tile([E, AD], BF16, tag="eb_nat")
    nc.vector.tensor_copy(eb_nat[:], eb_f32[:])

    ew_sb = pool.tile([128, 8, E], F32, tag="ew_sb")
    nc.sync.dma_start(ew_sb[:], ew.ap().rearrange("(t p) e -> p t e", p=128))
    ewr = pool.tile([128, 8, E], F32, tag="ewr")
    nc.vector.tensor_scalar_max(ewr[:], ew_sb[:], 0.0)

    ewt_sb = pool.tile([E, TOK], F32, tag="ewt_sb")
    nc.sync.dma_start(ewt_sb[:], ewt[:])
    ewrT_sb = pool.tile([E, TOK], BF16, tag="ewrT_sb")
    nc.vector.tensor_scalar_max(ewrT_sb[:], ewt_sb[:], 0.0)
    ones8 = pool.tile([E, 1], F32, tag="ones8")
    nc.vector.memset(ones8[:], 1.0)
    sumw_row = pool.tile([1, TOK], F32, tag="sumw_row")
    for n in range(NCH):
        psw = ps.tile([1, 512], F32, tag="ps")
        nc.tensor.matmul(psw[:], ones8[:], ewt_sb[:, n * 512:(n + 1) * 512],
                         start=True, stop=True)
        nc.vector.tensor_copy(sumw_row[:, n * 512:(n + 1) * 512], psw[:])
    sumwB = pool.tile([128, TOK], F32, tag="sumwB")
    nc.gpsimd.partition_broadcast(sumwB[:], sumw_row[:])

    # stationary weight banks
    wpost_sb = pool.tile([128, 16, AD], BF16, tag="wpost_sb")
    nc.sync.dma_start(wpost_sb[:], wpost_t.ap().rearrange("(k p) a -> p k a", p=128))
    wap_sb = pool.tile([128, 16, 128], BF16, tag="wap_sb")
    nc.sync.dma_start(wap_sb[:], wap_t.ap().rearrange("a (k h) -> a k h", h=128))
    wp_sb = pool.tile([128, 16, AD], BF16, tag="wp_sb")
    nc.sync.dma_start(wp_sb[:], wp.ap().rearrange("(k p) a -> p k a", p=128))
    at_sb = pool.tile([128, E, AD], BF16, tag="at_sb")
    nc.sync.dma_start(at_sb[:], a_t.ap().rearrange("e a c -> a e c"))

    # ---- LN helper (N-layout [128 tok, nb, 128 ad] blocks) ----
    def ln_stats(src, nb, tag):
        """Return (m, rs): per-token mean and 1/sqrt(var+eps) over the last
        (AD) dim of src [128, nb, AD]."""
        red = scr.tile([128, nb], F32, tag=tag + "_red")
        nc.vector.tensor_reduce(red[:], src[:], AX.X, OP.add)
        sq = scr.tile([128, nb, 128], F32, tag=tag + "_sq", bufs=1)
        nc.scalar.activation(sq[:], src[:], AF.Square)
        red2 = scr.tile([128, nb], F32, tag=tag + "_red2")
        nc.vector.tensor_reduce(red2[:], sq[:], AX.X, OP.add)
        # var+eps = (red2 - red^2/AD)/AD + eps
        t = scr.tile([128, nb], F32, tag=tag + "_t")
        nc.vector.tensor_tensor(t[:], red[:], red[:], OP.mult)
        v2 = scr.tile([128, nb], F32, tag=tag + "_v2")
        nc.vector.scalar_tensor_tensor(v2[:], t[:], -1.0 / AD, red2[:],
                                       OP.mult, OP.add)
        v3 = scr.tile([128, nb], F32, tag=tag + "_v3")
        nc.vector.tensor_scalar(v3[:], v2[:], 1.0 / AD, 1e-5, OP.mult, OP.add)
        sd = scr.tile([128, nb], F32, tag=tag + "_sd")
        nc.scalar.sqrt(sd[:], v3[:])
        rs = scr.tile([128, nb], F32, tag=tag + "_rs")
        nc.vector.reciprocal(rs[:], sd[:])
        m = scr.tile([128, nb], F32, tag=tag + "_m")
        nc.vector.tensor_scalar_mul(m[:], red[:], 1.0 / AD)
        return m, rs

    def layer_norm(src, nb, dst, tag, apply_gb=True):
        """dst[:, i, :] = LN(src[:, i, :]) (optionally * gB + bB)."""
        m, rs = ln_stats(src, nb, tag)
        for i in range(nb):
            if apply_gb:
                nrm = scr.tile([128, 128], F32, tag=tag + "_nrm")
                nc.vector.tensor_scalar(nrm[:], src[:, i, :], m[:, i:i + 1],
                                        rs[:, i:i + 1], OP.subtract, OP.mult)
                nrm2 = scr.tile([128, 128], F32, tag=tag + "_nrm2")
                nc.vector.tensor_tensor(nrm2[:], nrm[:], gB[:], OP.mult)
                nc.vector.tensor_tensor(dst[:, i, :], nrm2[:], bB[:], OP.add)
            else:
                nc.vector.tensor_scalar(dst[:, i, :], src[:, i, :],
                                        m[:, i:i + 1], rs[:, i:i + 1],
                                        OP.subtract, OP.mult)

    def transpose_blk(dst, src_ap, dtype, tpool=None):
        """dst[128,128] sbuf slice <- src_ap.T via PE (dtype BF16 or F32)."""
        pt = (tpool or ps).tile([128, 128], dtype,
                                tag="pt" if tpool is not None else "ps",
                                name="pt")
        nc.tensor.transpose(pt[:], src_ap, ident_b[:] if dtype == BF16 else ident_f[:])
        nc.vector.tensor_copy(dst, pt[:])

    # =================== P1: pre (own tokens, T-layout) ===================
    # ===== Wc = (0.1*Wo@Wp).T [AD, D] — emitted before up/gate so wo loads
    # prefetch early and the matmuls fill early PE gaps =====
    _mark(nc, "P5b_wc")
    wc = pool.tile([128, D], BF16, tag="wc")
    with tc.tile_pool(name="pwc", bufs=2, space="PSUM") as pwc_pool:
        pwcs = [pwc_pool.tile([128, 512], F32, tag="pwc", name=f"pwc{h}")
                for h in range(2)]
        for k in range(16):
            wo_k = wpool.tile([128, D], BF16, tag="wo_k")
            nc.gpsimd.dma_start(wo_k[:], wo_t.ap()[k * 128:(k + 1) * 128, :])
            for half in range(2):
                nc.tensor.matmul(pwcs[half][:], wp_sb[:, k, :],
                                 wo_k[:, half * 512:(half + 1) * 512],
                                 start=(k == 0), stop=(k == 15))
        for half in range(2):
            nc.vector.tensor_copy(wc[:, half * 512:(half + 1) * 512],
                                  pwcs[half][:])

    _mark(nc, "P1_pre")
    preT = pool.tile([128, TOK], BF16, tag="preT")
    for n in range(NCH):
        pp = ps.tile([128, 512], F32, tag="ps")
        for k in range(8):
            nc.tensor.matmul(pp[:], wpre_sb[:, k, :], xt_sb[:, k, n * 512:(n + 1) * 512],
                             start=(k == 0), stop=(k == 7))
        nc.scalar.activation(preT[:, n * 512:(n + 1) * 512], pp[:],
                             AF.Identity, bias=bpre_c[:])

    # =================== P2: adapt_in (own) ===================
    _mark(nc, "P2_lnin")
    preN = pool.tile([128, 8, AD], BF16, tag="preN")
    for i in range(8):
        transpose_blk(preN[:, i, :], preT[:, i * 128:(i + 1) * 128], BF16)
    ainN = pool.tile([128, 8, AD], BF16, tag="ainN")
    layer_norm(preN, 8, ainN, "lnin")
    ainT = pool.tile([128, TOK], BF16, tag="ainT")
    for i in range(8):
        transpose_blk(ainT[:, i * 128:(i + 1) * 128], ainN[:, i, :], BF16)
    # AllGather of adapt_in fires immediately (hidden under up/gate)
    cc_in1 = dram.tile([TOK, AD], BF16, tag="cc_in1")
    cc_out1 = dram.tile([2 * TOK, AD], BF16, tag="cc_out1")
    nc.sync.dma_start(
        cc_in1[:].rearrange("(t p) a -> p t a", p=128), ainN[:])
    if fake_cc:
        nc.sync.dma_start(cc_out1[0:TOK, :], cc_in1[:])
        nc.sync.dma_start(cc_out1[TOK:2 * TOK, :], cc_in1[:])
    else:
        nc.gpsimd.collective_compute(
            "AllGather", OP.bypass,
            replica_groups=[[0, 1], [2, 3], [4, 5], [6, 7]],
            ins=[cc_in1[:].opt()], outs=[cc_out1[:].opt()])
    ainN_f = pool.tile([128, 16, AD], BF16, tag="ainN_f")
    nc.sync.dma_start(ainN_f[:, 0:8, :],
                      cc_out1[0:TOK, :].rearrange("(t p) a -> p t a", p=128))
    nc.sync.dma_start(ainN_f[:, 8:16, :],
                      cc_out1[TOK:2 * TOK, :].rearrange("(t p) a -> p t a", p=128))

    # =================== P3: up/gate -> hiddenT, wpost accum ===================
    _mark(nc, "P3_upgate")
    hT = pool.tile([128, 16, TOK], BF16, tag="hT")
    ppo_pool = tc.alloc_tile_pool(name="ppo_pool", bufs=2, space="PSUM")
    ppo = [ppo_pool.tile([128, 512], F32, tag="ppo", name=f"ppo{n}") for n in range(NCH)]
    with tc.tile_pool(name="pug", bufs=2, space="PSUM") as pug:
        for ht in range(16):
            wu_ht = wpool.tile([128, 8, 128], BF16, tag="wu_ht")
            wg_ht = wpool.tile([128, 8, 128], BF16, tag="wg_ht")
            nc.sync.dma_start(wu_ht[:], wu_t.ap()[ht])
            nc.sync.dma_start(wg_ht[:], wg_t.ap()[ht])
            for n in range(NCH):
                pu = pug.tile([128, 512], F32, tag="pu")
                pg = pug.tile([128, 512], F32, tag="pg")
                for k in range(8):
                    nc.tensor.matmul(pu[:], wu_ht[:, k, :],
                                     xt_sb[:, k, n * 512:(n + 1) * 512],
                                     start=(k == 0), stop=(k == 7))
                for k in range(8):
                    nc.tensor.matmul(pg[:], wg_ht[:, k, :],
                                     xt_sb[:, k, n * 512:(n + 1) * 512],
                                     start=(k == 0), stop=(k == 7))
                silg = scr.tile([128, 512], F32, tag="silg")
                nc.scalar.activation(silg[:], pg[:], AF.Silu,
                                     bias=bgt[:, ht:ht + 1])
                nc.vector.scalar_tensor_tensor(
                    hT[:, ht, n * 512:(n + 1) * 512], pu[:], but[:, ht:ht + 1],
                    silg[:], OP.add, OP.mult)
                # wpost matmul for the PREVIOUS ht (software pipeline: keeps
                # the PE stream from stalling on this iteration's DVE result)
                if ht > 0:
                    nc.tensor.matmul(ppo[n][:], wpost_sb[:, ht - 1, :],
                                     hT[:, ht - 1, n * 512:(n + 1) * 512],
                                     start=(ht == 1), stop=False)
        for n in range(NCH):
            nc.tensor.matmul(ppo[n][:], wpost_sb[:, 15, :],
                             hT[:, 15, n * 512:(n + 1) * 512],
                             start=False, stop=True)

    # =================== P4a: adapt_out (own) + collective ===================
    _mark(nc, "P4a_aout")
    postT = pool.tile([128, TOK], BF16, tag="postT")
    for n in range(NCH):
        nc.scalar.activation(postT[:, n * 512:(n + 1) * 512], ppo[n][:],
                             AF.Identity, bias=bpost_c[:])
    ppo_pool.release()
    # Allocate P6's psum pools NOW (zones from the released up/gate banks) so
    # P6 never acquires a release-dependency on the expert branch's psum.
    pad_pool = tc.alloc_tile_pool(name="pad_pool", bufs=2, space="PSUM")
    paw_pool = tc.alloc_tile_pool(name="paw_pool", bufs=2, space="PSUM")
    postN = pool.tile([128, 8, AD], BF16, tag="postN")
    for i in range(8):
        transpose_blk(postN[:, i, :], postT[:, i * 128:(i + 1) * 128], BF16)
    aoutN = pool.tile([128, 8, AD], BF16, tag="aoutN")
    layer_norm(postN, 8, aoutN, "lnout", apply_gb=False)

    cc_in2 = dram.tile([TOK, AD], BF16, tag="cc_in2")
    cc_out2 = dram.tile([2 * TOK, AD], BF16, tag="cc_out2")
    nc.sync.dma_start(
        cc_in2[:].rearrange("(t p) a -> p t a", p=128), aoutN[:])
    if fake_cc:
        nc.sync.dma_start(cc_out2[0:TOK, :], cc_in2[:])
        nc.sync.dma_start(cc_out2[TOK:2 * TOK, :], cc_in2[:])
    else:
        nc.gpsimd.collective_compute(
            "AllGather", OP.bypass,
            replica_groups=[[0, 1], [2, 3], [4, 5], [6, 7]],
            ins=[cc_in2[:].opt()], outs=[cc_out2[:].opt()])

    # =================== P4b: collective readback ===================
    _mark(nc, "P4b_read")
    aoutN_f = pool.tile([128, 16, AD], BF16, tag="aoutN_f")
    nc.sync.dma_start(aoutN_f[:, 0:8, :],
                      cc_out2[0:TOK, :].rearrange("(t p) a -> p t a", p=128))
    nc.sync.dma_start(aoutN_f[:, 8:16, :],
                      cc_out2[TOK:2 * TOK, :].rearrange("(t p) a -> p t a", p=128))
    aoutT = pool.tile([128, S], BF16, tag="aoutT")
    # ln_g/ln_b were skipped before the collective; fused into the per-block
    # transpose copy here (per-partition scalars in T layout).
    for t in range(16):
        pt = ps.tile([128, 128], BF16, tag="ps", name="pt")
        nc.tensor.transpose(pt[:], aoutN_f[:, t, :], ident_b[:])
        nc.vector.tensor_scalar(aoutT[:, t * 128:(t + 1) * 128], pt[:],
                                lng_c[:], lnb_c[:], OP.mult, OP.add)

    # =================== P5: expert branch (independent of collective) =======
    # hw[s,a] = sum_e w_e[s] * (LN_e(h_e)[s,a]*eg[e,a]); the w_e*eb[e,a] term
    # is folded into the hwT transpose psum as a rank-8 matmul. Expert matmul
    # results are copied to SBUF immediately (ACT, normal priority) so the
    # psum ring turns over fast; the LN chains for all 8 experts then run
    # concurrently at low priority, filling engine-idle gaps.
    _mark(nc, "P5_expert")
    hw = pool.tile([128, 8, AD], F32, tag="hw")
    hw2 = pool.tile([128, 8, AD], F32, tag="hw2")
    ph_sb = pool.tile([128, E, 8, AD], BF16, tag="ph_sb")
    with tc.tile_pool(name="pexp", bufs=2, space="PSUM") as pexp:
        for e in range(E):
            phs = [pexp.tile([128, 4, AD], F32, tag="ph", name=f"ph{e}_{hb}")
                   for hb in range(2)]
            for i in range(8):
                nc.tensor.matmul(phs[i // 4][:, i % 4, :],
                                 preT[:, i * 128:(i + 1) * 128],
                                 at_sb[:, e, :], start=True, stop=True)
            for hb in range(2):
                nc.scalar.activation(ph_sb[:, e, hb * 4:(hb + 1) * 4, :],
                                     phs[hb][:], AF.Copy)
    lowprio = tc.high_priority(offset=-1000000)
    lowprio.__enter__()
    for e in range(E):
        src = ph_sb[:, e, :, :]
        red = scr.tile([128, 8], F32, tag="x_red")
        nc.vector.tensor_reduce(red[:], src, AX.X, OP.add)
        sq = scr.tile([128, 8, AD], BF16, tag="x_sq", bufs=1)
        nc.scalar.activation(sq[:], src, AF.Square)
        red2 = scr.tile([128, 8], F32, tag="x_red2")
        nc.vector.tensor_reduce(red2[:], sq[:], AX.X, OP.add)
        t = scr.tile([128, 8], F32, tag="x_t")
        nc.vector.tensor_tensor(t[:], red[:], red[:], OP.mult)
        v2 = scr.tile([128, 8], F32, tag="x_v2")
        nc.vector.scalar_tensor_tensor(v2[:], t[:], -1.0 / AD, red2[:],
                                       OP.mult, OP.add)
        v3 = scr.tile([128, 8], F32, tag="x_v3")
        nc.vector.tensor_scalar(v3[:], v2[:], 1.0 / AD, 1e-5, OP.mult, OP.add)
        sd = scr.tile([128, 8], F32, tag="x_sd")
        nc.scalar.sqrt(sd[:], v3[:])
        rs = scr.tile([128, 8], F32, tag="x_rs")
        nc.vector.reciprocal(rs[:], sd[:])
        rsw = scr.tile([128, 8], F32, tag="x_rsw")
        nc.vector.tensor_tensor(rsw[:], rs[:], ewr[:, :, e], OP.mult)
        nmrsw = scr.tile([128, 8], F32, tag="x_nmrsw")
        nc.vector.scalar_tensor_tensor(nmrsw[:], red[:], -1.0 / AD, rsw[:],
                                       OP.mult, OP.mult)
        nrmall = scr.tile([128, 8, AD], F32, tag="x_nrm", bufs=1)
        for blk in range(8):
            nc.scalar.activation(nrmall[:, blk, :], ph_sb[:, e, blk, :],
                                 AF.Identity, scale=rsw[:, blk:blk + 1],
                                 bias=nmrsw[:, blk:blk + 1])
        egv = egB[:, e, :].unsqueeze(1).broadcast_to([128, 8, AD])
        acc = hw if e % 2 == 0 else hw2
        if e < 2:
            nc.vector.tensor_tensor(acc[:], nrmall[:], egv, OP.mult)
        else:
            t2 = scr.tile([128, 8, AD], F32, tag="x_t2", bufs=1)
            nc.vector.tensor_tensor(t2[:], nrmall[:], egv, OP.mult)
            nc.vector.tensor_tensor(acc[:], t2[:], acc[:], OP.add)
    nc.vector.tensor_tensor(hw[:], hw[:], hw2[:], OP.add)
    lowprio.__exit__(None, None, None)
    hwT = pool.tile([128, TOK], BF16, tag="hwT")
    for half in range(2):
        pt = ps.tile([128, 512], F32, tag="ps", name=f"hwt{half}")
        nc.tensor.matmul(pt[:], eb_nat[:],
                         ewrT_sb[:, half * 512:(half + 1) * 512],
                         start=True, stop=False)
        for q in range(4):
            blk = half * 4 + q
            nc.tensor.matmul(pt[:, q * 128:(q + 1) * 128], hw[:, blk, :],
                             ident_f[:], is_transpose=True,
                             start=False, stop=(q == 3))
        nc.vector.tensor_copy(hwT[:, half * 512:(half + 1) * 512], pt[:])

    # =================== P6: aw + adapt ===================
    _mark(nc, "P6_adapt")
    pad = [pad_pool.tile([128, 512], F32, tag="pad", name=f"pad{n}")
           for n in range(NCH)]
    if True:
        # software pipeline: adapt matmul for step t emitted during step t+1,
        # so the PE stream never waits on the clip/silu chain.
        aw_tiles = {}
        for t in range(16):
            for n in range(NCH):
                paw = paw_pool.tile([128, 512], F32, tag="paw")
                nc.tensor.matmul(paw[:], aoutT[:, t * 128:(t + 1) * 128],
                                 ainT[:, n * 512:(n + 1) * 512],
                                 start=True, stop=True)
                cl = scr.tile([128, 512], F32, tag="cl", bufs=2)
                nc.vector.tensor_scalar(cl[:], paw[:], 5.0, -5.0, OP.min, OP.max)
                aw_bf = scr.tile([128, 512], BF16, tag="aw_bf", bufs=3)
                nc.scalar.activation(aw_bf[:], cl[:], AF.Silu)
                aw_tiles[(t, n)] = aw_bf
                if t > 0:
                    nc.tensor.matmul(pad[n][:], ainN_f[:, t - 1, :],
                                     aw_tiles.pop((t - 1, n))[:],
                                     start=(t == 1), stop=False)
        for n in range(NCH):
            nc.tensor.matmul(pad[n][:], ainN_f[:, 15, :],
                             aw_tiles.pop((15, n))[:], start=False, stop=True)
    adT = pool.tile([128, TOK], BF16, tag="adT")
    for n in range(NCH):
        nc.vector.tensor_copy(adT[:, n * 512:(n + 1) * 512], pad[n][:])
    paw_pool.release()
    pad_pool.release()

    # =================== P7: hidden += 0.1 * adapt @ Wap.T ===================
    _mark(nc, "P7_wap")
    # P7 (hidden += adapt@WapT) and P8 (shared+combine+out) interleaved per
    # 512-token half: P8 for half n only needs hT columns n updated, so its
    # matmuls overlap the other half's P7 DVE adds.
    with tc.tile_pool(name="p7", bufs=2, space="PSUM") as p7, \
         tc.tile_pool(name="psh", bufs=4, space="PSUM") as psh_pool:
        for n in range(NCH):
            c0, c1 = n * 512, (n + 1) * 512
            for ht in range(16):
                pwap = p7.tile([128, 512], F32, tag="p7")
                nc.tensor.matmul(pwap[:], wap_sb[:, ht, :], adT[:, c0:c1],
                                 start=True, stop=True)
                nc.vector.tensor_tensor(hT[:, ht, c0:c1], pwap[:],
                                        hT[:, ht, c0:c1], OP.add)
            if n == 0:
                _mark(nc, "P8_out")

            def _finish(dt, psh):
                pct = ps.tile([128, 512], F32, tag="ps", name="pct")
                nc.tensor.matmul(pct[:], wc[:, dt * 128:(dt + 1) * 128],
                                 hwT[:, c0:c1], start=True, stop=True)
                tcomb = scr.tile([128, 512], F32, tag="tcomb", name="tcomb")
                nc.vector.scalar_tensor_tensor(
                    tcomb[:], psh[:], bdt[:, dt:dt + 1], sumwB[:, c0:c1],
                    OP.add, OP.mult)
                osb = scr.tile([128, 512], F32, tag="osb", name="osb")
                nc.vector.tensor_tensor(osb[:], tcomb[:], pct[:], OP.add)
                nc.gpsimd.dma_start(out.ap()[dt * 128:(dt + 1) * 128, c0:c1],
                                    osb[:])

            prev = None
            for dt in range(8):
                wd_dt = wpool.tile([128, 16, 128], BF16, tag="wd_dt")
                nc.sync.dma_start(wd_dt[:], wd_t.ap()[dt])
                psh = psh_pool.tile([128, 512], F32, tag="psh")
                for k in range(16):
                    nc.tensor.matmul(psh[:], wd_dt[:, k, :], hT[:, k, c0:c1],
                                     start=(k == 0), stop=(k == 15))
                if prev is not None:
                    _finish(*prev)
                prev = (dt, psh)
            _finish(*prev)

    stack.close()


def _prep_inputs(inputs):
    f = {k: np.asarray(v, np.float32) for k, v in inputs.items()}

    def tbf(a):  # transpose + bf16, contiguous
        return np.ascontiguousarray(a.T).astype(BF)

    def swz(wt, nb):  # [K, M] -> [M/128, 128(p of K), K/128, 128] tiles
        k, mdim = wt.shape
        a = wt.reshape(k // 128, 128, nb, 128)
        return np.ascontiguousarray(a.transpose(2, 1, 0, 3)).astype(BF)

    shared = {
        "wu_t": swz(np.ascontiguousarray(f["Wu"].T), 16),
        "wg_t": swz(np.ascontiguousarray(f["Wg"].T), 16),
        "wd_t": swz(np.ascontiguousarray(f["Wd"].T), 8),
        "wo_t": tbf(f["Wo"]), "wpre_t": tbf(f["Wpre"]), "wpost_t": tbf(f["Wpost"]),
        "wap_t": tbf(0.1 * f["Wap"]), "wp": (0.1 * f["Wp"]).astype(BF),
        "a_t": np.ascontiguousarray(f["A"].transpose(0, 2, 1)).astype(BF),
        "bu": f["bu"], "bg": f["bg"], "bd": f["bd"],
        "bpre": f["bpre"], "bpost": f["bpost"],
        "ln_g": f["ln_g"], "ln_b": f["ln_b"], "eg": f["eg"], "eb": f["eb"],
        "id_bf": np.eye(128, dtype=np.float32).astype(BF),
        "id_f32": np.eye(128, dtype=np.float32),
    }
    in_maps = []
    for c in range(N_CORES):
        b, j = c // 2, c % 2
        sl = slice(j * TOK, (j + 1) * TOK)
        m = dict(shared)
        m["xt"] = tbf(f["x"][b, sl, :])
        m["ew"] = np.ascontiguousarray(f["expert_weights"][b, sl, :])
        m["ewt"] = np.ascontiguousarray(f["expert_weights"][b, sl, :].T)
        in_maps.append(m)
    return in_maps


def kernel(**inputs):
    global _NC_CACHE
    if _NC_CACHE is None:
        _NC_CACHE = build()
    in_maps = _prep_inputs(inputs)
    res = bass_utils.run_bass_kernel_spmd(
        _NC_CACHE, in_maps, core_ids=list(range(N_CORES)))
    out = np.empty((B, S, D), np.float32)
    for c in range(N_CORES):
        b, j = c // 2, c % 2
        out[b, j * TOK:(j + 1) * TOK, :] = res.results[c]["out"].T
    return out


